# revision 3
# baseline (speedup 1.0000x reference)
"""BertSelfAttention (with group_prob scaling + mask|diag masking) on 8 Trainium2 cores.

Strategy: data-parallel over batch (16 batches -> 2 per core). Each core computes,
for its 2 batches:
  qT/kT = (Wq/8 | Wk) @ hs^T            (fp32r matmuls, PSUM fp32 accumulate)
  v     = hs @ Wv^T + bv
  per (head, row-block): scores = qT_h^T @ kT_h  (K=64)
    masked = scores + M     (M = 0 / -inf additive mask, exact -inf via DVE add)
    p = exp(masked)         (ACT, with accumulated row-sum)
    pgT = transpose(p) * gpT (PE transpose + fused DVE multiply)
    ctx = (pgT^T @ v_h) * (1/sum)
Host side does layout prep only: transposes (hs^T, W^T, gp^T), folding the
1/sqrt(dh)=1/8 scale into Wq/bq (exact, power of two), and building the additive
mask M = where(mask|I, 0, -inf).
"""

import os
import sys

import numpy as np

for _p in ("/opt/trn_rl_repo", "/root/.axon_site/_ro/trn_rl_repo"):
    if _p not in sys.path and os.path.isdir(_p):
        sys.path.append(_p)

import concourse.bacc as bacc
import concourse.tile as tile
from concourse import mybir
from concourse.bass_utils import run_bass_kernel_spmd
from concourse.masks import make_identity
import concourse.bass as bass

NB = 2          # batches per core
S = 512         # sequence length
H = 1024        # hidden
NH = 16         # heads
DH = 64         # head dim
NCORES = 8

F32 = mybir.dt.float32
F32R = mybir.dt.float32r

# dtype used for matmul operands (fp32 bits; f32r streams at full PE rate)
def _r(ap):
    return ap.bitcast(F32R)


def build_nc(transpose_f32r=False):
    nc = bacc.Bacc("TRN2", target_bir_lowering=False, debug=False)
    AF = mybir.ActivationFunctionType

    hsT_d = nc.dram_tensor("hsT", [NB, H, S], F32R, kind="ExternalInput").ap()
    wqT_d = nc.dram_tensor("wqT", [H, H], F32R, kind="ExternalInput").ap()
    wkT_d = nc.dram_tensor("wkT", [H, H], F32R, kind="ExternalInput").ap()
    wvT_d = nc.dram_tensor("wvT", [H, H], F32R, kind="ExternalInput").ap()
    bq_d = nc.dram_tensor("bq", [H], F32, kind="ExternalInput").ap()
    bk_d = nc.dram_tensor("bk", [H], F32, kind="ExternalInput").ap()
    bv_d = nc.dram_tensor("bv", [H], F32, kind="ExternalInput").ap()
    m_d = nc.dram_tensor("madd", [NB, S, S], F32, kind="ExternalInput").ap()
    gpT_d = nc.dram_tensor("gpT", [NB, S, S], F32, kind="ExternalInput").ap()
    scores_d = nc.dram_tensor("scores", [NB, NH, S, S], F32, kind="ExternalOutput").ap()
    ctx_d = nc.dram_tensor("ctx", [NB, S, H], F32, kind="ExternalOutput").ap()

    HC = H // 128   # 8 h chunks
    SB = S // 128   # 4 s blocks

    with tile.TileContext(nc) as tc:
        with (
            tc.tile_pool(name="wpool", bufs=1) as wpool,
            tc.tile_pool(name="bpool", bufs=1) as bpool,
            tc.tile_pool(name="perb", bufs=1) as perb,
            tc.tile_pool(name="work", bufs=1) as work,
            tc.tile_pool(name="psA", bufs=3, space="PSUM") as psA,
            tc.tile_pool(name="psT", bufs=2, space="PSUM") as psT,
            tc.tile_pool(name="psC", bufs=2, space="PSUM") as psC,
        ):
            # ---- one-time loads -------------------------------------------------
            wq_sb = wpool.tile([128, HC, H], F32R, tag="wq")
            wk_sb = wpool.tile([128, HC, H], F32R, tag="wk")
            wv_sb = wpool.tile([128, HC, H], F32R, tag="wv")
            nc.sync.dma_start(out=wq_sb, in_=wqT_d.rearrange("(c p) o -> p c o", p=128))
            nc.sync.dma_start(out=wk_sb, in_=wkT_d.rearrange("(c p) o -> p c o", p=128))
            nc.sync.dma_start(out=wv_sb, in_=wvT_d.rearrange("(c p) o -> p c o", p=128))

            ident = bpool.tile([128, 128], F32, tag="ident")
            make_identity(nc, ident)
            tdt = F32R if transpose_f32r else F32
            ident_t = ident.bitcast(tdt)

            bq_sb = bpool.tile([128, HC], F32, tag="bq")
            bk_sb = bpool.tile([128, HC], F32, tag="bk")
            nc.sync.dma_start(out=bq_sb, in_=bq_d.rearrange("(c p) -> p c", p=128))
            nc.sync.dma_start(out=bk_sb, in_=bk_d.rearrange("(c p) -> p c", p=128))
            bvb_sb = bpool.tile([128, H], F32, tag="bvb")
            bv_bcast = bass.AP(tensor=bv_d.tensor, offset=bv_d.offset,
                               ap=[[0, 128]] + list(bv_d.ap))
            nc.sync.dma_start(out=bvb_sb, in_=bv_bcast)

            for b in range(NB):
                # ---- per-batch loads -------------------------------------------
                hsT_sb = perb.tile([128, HC, S], F32R, tag="hsT")
                nc.sync.dma_start(
                    out=hsT_sb, in_=hsT_d[b].rearrange("(c p) s -> p c s", p=128))
                m_sb = perb.tile([128, SB, S], F32, tag="m")
                nc.sync.dma_start(
                    out=m_sb, in_=m_d[b].rearrange("(r p) j -> p r j", p=128))
                gpT_sb = perb.tile([128, SB, S], F32, tag="gpT")
                nc.sync.dma_start(
                    out=gpT_sb, in_=gpT_d[b].rearrange("(c p) i -> p c i", p=128))

                # ---- projections -----------------------------------------------
                qT_sb = perb.tile([128, HC, S], F32R, tag="qT")
                kT_sb = perb.tile([128, HC, S], F32R, tag="kT")
                for (w_sb, b_sb, o_sb) in ((wq_sb, bq_sb, qT_sb), (wk_sb, bk_sb, kT_sb)):
                    for co in range(HC):
                        ps = psA.tile([128, S], F32, tag="ps", name="ps_proj")
                        for ci in range(HC):
                            nc.tensor.matmul(
                                ps, w_sb[:, ci, co * 128:(co + 1) * 128],
                                hsT_sb[:, ci, :],
                                start=(ci == 0), stop=(ci == HC - 1))
                        nc.scalar.activation(
                            o_sb[:, co, :], ps, AF.Identity,
                            bias=b_sb[:, co:co + 1])
                v_sb = perb.tile([128, SB, H], F32R, tag="v")
                for sb_i in range(SB):
                    for half in range(2):
                        ps = psA.tile([128, S], F32, tag="ps", name="ps_v")
                        for ci in range(HC):
                            nc.tensor.matmul(
                                ps, hsT_sb[:, ci, sb_i * 128:(sb_i + 1) * 128],
                                wv_sb[:, ci, half * 512:(half + 1) * 512],
                                start=(ci == 0), stop=(ci == HC - 1))
                        nc.vector.tensor_add(
                            v_sb[:, sb_i, half * 512:(half + 1) * 512], ps,
                            bvb_sb[:, half * 512:(half + 1) * 512])

                # ---- attention -------------------------------------------------
                for r in range(SB):
                    ctx_sb = work.tile([128, H], F32, tag="ctx", bufs=2, name="ctx_sb")
                    for h in range(NH):
                        hc, hp = h // 2, (h % 2) * 64
                        q_l = qT_sb[hp:hp + 64, hc, r * 128:(r + 1) * 128]
                        k_l = kT_sb[hp:hp + 64, hc, :]
                        ps_s = psA.tile([128, S], F32, tag="ps", name="ps_s")
                        nc.tensor.matmul(ps_s, q_l, k_l, start=True, stop=True)
                        masked = work.tile([128, S], F32, tag="masked", bufs=3,
                                           name="masked")
                        nc.vector.tensor_add(masked, ps_s, m_sb[:, r, :])
                        nc.sync.dma_start(
                            out=scores_d[b, h, r * 128:(r + 1) * 128, :], in_=masked)
                        p_sb = work.tile([128, S], F32, tag="p", bufs=3, name="p_sb")
                        ssum = work.tile([128, 1], F32, tag="ssum", bufs=4, name="ssum")
                        nc.scalar.activation(p_sb, masked, AF.Exp, accum_out=ssum)
                        rs = work.tile([128, 1], F32, tag="rs", bufs=4, name="rs")
                        nc.vector.reciprocal(rs, ssum)
                        ps_t = psT.tile([128, S], F32, tag="pst", name="ps_t")
                        for c in range(SB):
                            nc.tensor.transpose(
                                ps_t[:, c * 128:(c + 1) * 128],
                                p_sb[:, c * 128:(c + 1) * 128],
                                ident)
                        pgT = work.tile([128, SB, 128], F32R, tag="pgT", bufs=3,
                                        name="pgT")
                        nc.vector.tensor_mul(
                            pgT, ps_t.rearrange("p (c i) -> p c i", c=SB),
                            gpT_sb[:, :, r * 128:(r + 1) * 128])
                        ps_c = psC.tile([128, DH], F32, tag="psc", name="ps_c")
                        for c in range(SB):
                            nc.tensor.matmul(
                                ps_c, pgT[:, c, :],
                                v_sb[:, c, h * DH:(h + 1) * DH],
                                start=(c == 0), stop=(c == SB - 1))
                        nc.vector.tensor_scalar(
                            out=ctx_sb[:, h * DH:(h + 1) * DH], in0=ps_c,
                            scalar1=rs, scalar2=None, op0=mybir.AluOpType.mult)
                    nc.sync.dma_start(
                        out=ctx_d[b, r * 128:(r + 1) * 128, :], in_=ctx_sb)
    nc.compile()
    return nc


_NC_CACHE = {}


def _get_nc():
    if "nc" not in _NC_CACHE:
        _NC_CACHE["nc"] = build_nc()
    return _NC_CACHE["nc"]


def prep_inputs(hidden_states, attention_mask, group_prob, Wq, bq, Wk, bk, Wv, bv):
    """Host-side layout prep + sharding. Returns list of 8 per-core input maps."""
    f = np.float32
    hs = np.asarray(hidden_states, dtype=f)
    B = hs.shape[0]
    hsT = np.ascontiguousarray(hs.transpose(0, 2, 1))
    wqT = np.ascontiguousarray(np.asarray(Wq, dtype=f).T / 8.0)
    wkT = np.ascontiguousarray(np.asarray(Wk, dtype=f).T)
    wvT = np.ascontiguousarray(np.asarray(Wv, dtype=f).T)
    bq8 = np.asarray(bq, dtype=f) / 8.0
    bk_ = np.asarray(bk, dtype=f)
    bv_ = np.asarray(bv, dtype=f)
    keep = (np.asarray(attention_mask)[:, 0] != 0) | np.eye(S, dtype=bool)
    madd = np.where(keep, f(0.0), f(-np.inf)).astype(f)
    gpT = np.ascontiguousarray(np.asarray(group_prob, dtype=f).transpose(0, 2, 1))
    in_maps = []
    for i in range(NCORES):
        sl = slice(i * NB, (i + 1) * NB)
        in_maps.append({
            "hsT": hsT[sl], "wqT": wqT, "wkT": wkT, "wvT": wvT,
            "bq": bq8, "bk": bk_, "bv": bv_,
            "madd": madd[sl], "gpT": gpT[sl],
        })
    return in_maps


def kernel(hidden_states, attention_mask, group_prob, Wq, bq, Wk, bk, Wv, bv):
    in_maps = prep_inputs(hidden_states, attention_mask, group_prob,
                          Wq, bq, Wk, bk, Wv, bv)
    nc = _get_nc()
    res = run_bass_kernel_spmd(nc, in_maps, core_ids=list(range(NCORES)))
    ctx = np.concatenate([res.results[i]["ctx"] for i in range(NCORES)], axis=0)
    scores = np.concatenate([res.results[i]["scores"] for i in range(NCORES)], axis=0)
    return ctx, scores


# revision 4
# speedup vs baseline: 1.1605x; 1.1605x over previous
"""BertSelfAttention (group_prob-scaled probs, mask|diag masking) on 8 TRN2 cores.

Sharding: data-parallel over batch (16 -> 2 per core). Device math per (b, head):
  qT/kT = W^T-layout projections of hs^T (f32r matmuls, full PE rate)
  scores = qT_h^T @ kT_h   (1/8 folded into Wq on host)
  masked = scores + M      (M in {0, -inf}; exact -inf via DVE add)
  p = exp(masked) [bf16] with per-row accumulated sum (ACT)
  pgT = transpose(p) * gpT [bf16]  (PE transpose + fused DVE multiply)
  ctx_h = (pgT^T @ v_h) / rowsum   (bf16 matmul, fp32 accumulate, ACT scale-copy)
Host side is layout-only: transposes (hs^T, W^T, gp^T), 1/8 scale fold (exact,
power of two), additive mask build, bf16 casts for the probability path.
"""

import os
import sys

import numpy as np

for _p in ("/opt/trn_rl_repo", "/root/.axon_site/_ro/trn_rl_repo"):
    if _p not in sys.path and os.path.isdir(_p):
        sys.path.append(_p)

import ml_dtypes
import concourse.bacc as bacc
import concourse.bass as bass
import concourse.tile as tile
from concourse import mybir
from concourse.bass_utils import run_bass_kernel_spmd
from concourse.masks import make_identity

NB = 2          # batches per core
S = 512         # sequence length
H = 1024        # hidden
NH = 16         # heads
DH = 64         # head dim
NCORES = 8
HC = H // 128   # 8 hidden chunks
SB = S // 128   # 4 seq blocks

F32 = mybir.dt.float32
F32R = mybir.dt.float32r
BF16 = mybir.dt.bfloat16


def build_nc():
    nc = bacc.Bacc("TRN2", target_bir_lowering=False, debug=False)
    AF = mybir.ActivationFunctionType

    hsT_d = nc.dram_tensor("hsT", [NB, H, S], F32R, kind="ExternalInput").ap()
    wqT_d = nc.dram_tensor("wqT", [H, H], F32R, kind="ExternalInput").ap()
    wkT_d = nc.dram_tensor("wkT", [H, H], F32R, kind="ExternalInput").ap()
    wvT_d = nc.dram_tensor("wvT", [H, H], F32R, kind="ExternalInput").ap()
    bq_d = nc.dram_tensor("bq", [H], F32, kind="ExternalInput").ap()
    bk_d = nc.dram_tensor("bk", [H], F32, kind="ExternalInput").ap()
    bv_d = nc.dram_tensor("bv", [H], F32, kind="ExternalInput").ap()
    m_d = nc.dram_tensor("madd", [NB, S, S], BF16, kind="ExternalInput").ap()
    gpT_d = nc.dram_tensor("gpT", [NB, S, S], BF16, kind="ExternalInput").ap()
    scores_d = nc.dram_tensor("scores", [NB, NH, S, S], F32, kind="ExternalOutput").ap()
    ctx_d = nc.dram_tensor("ctx", [NB, S, H], F32, kind="ExternalOutput").ap()

    with tile.TileContext(nc) as tc:
        with (
            tc.tile_pool(name="wpool", bufs=1) as wpool,
            tc.tile_pool(name="bpool", bufs=1) as bpool,
            tc.tile_pool(name="perb", bufs=1) as perb,
            tc.tile_pool(name="work", bufs=1) as work,
            tc.tile_pool(name="psA", bufs=3, space="PSUM") as psA,
            tc.tile_pool(name="psT", bufs=2, space="PSUM") as psT,
            tc.tile_pool(name="psC", bufs=2, space="PSUM") as psC,
        ):
            # ---- one-time loads ------------------------------------------------
            wq_sb = wpool.tile([128, HC, H], F32R, tag="wq")
            wk_sb = wpool.tile([128, HC, H], F32R, tag="wk")
            wv_sb = wpool.tile([128, HC, H], F32R, tag="wv")
            nc.sync.dma_start(out=wq_sb, in_=wqT_d.rearrange("(c p) o -> p c o", p=128))
            nc.sync.dma_start(out=wk_sb, in_=wkT_d.rearrange("(c p) o -> p c o", p=128))
            nc.sync.dma_start(out=wv_sb, in_=wvT_d.rearrange("(c p) o -> p c o", p=128))

            ident = bpool.tile([128, 128], F32, tag="ident")
            make_identity(nc, ident)
            ident_bf = bpool.tile([128, 128], BF16, tag="ident_bf")
            nc.vector.tensor_copy(ident_bf, ident)

            bq_sb = bpool.tile([128, HC], F32, tag="bq")
            bk_sb = bpool.tile([128, HC], F32, tag="bk")
            nc.sync.dma_start(out=bq_sb, in_=bq_d.rearrange("(c p) -> p c", p=128))
            nc.sync.dma_start(out=bk_sb, in_=bk_d.rearrange("(c p) -> p c", p=128))
            bvb_sb = bpool.tile([128, H], F32, tag="bvb")
            bv_bcast = bass.AP(tensor=bv_d.tensor, offset=bv_d.offset,
                               ap=[[0, 128]] + list(bv_d.ap))
            nc.sync.dma_start(out=bvb_sb, in_=bv_bcast)

            for b in range(NB):
                # ---- per-batch loads ------------------------------------------
                hsT_sb = perb.tile([128, HC, S], F32R, tag="hsT")
                nc.sync.dma_start(
                    out=hsT_sb, in_=hsT_d[b].rearrange("(c p) s -> p c s", p=128))
                m_sb = perb.tile([128, SB, S], BF16, tag="m")
                nc.sync.dma_start(
                    out=m_sb, in_=m_d[b].rearrange("(r p) j -> p r j", p=128))
                gpT_sb = perb.tile([128, SB, S], BF16, tag="gpT")
                nc.sync.dma_start(
                    out=gpT_sb, in_=gpT_d[b].rearrange("(c p) i -> p c i", p=128))

                # ---- projections ----------------------------------------------
                qT_sb = perb.tile([128, HC, S], F32R, tag="qT")
                kT_sb = perb.tile([128, HC, S], F32R, tag="kT")
                for (w_sb, b_sb, o_sb) in ((wq_sb, bq_sb, qT_sb), (wk_sb, bk_sb, kT_sb)):
                    for co in range(HC):
                        ps = psA.tile([128, S], F32, tag="ps", name="ps_proj")
                        for ci in range(HC):
                            nc.tensor.matmul(
                                ps, w_sb[:, ci, co * 128:(co + 1) * 128],
                                hsT_sb[:, ci, :],
                                start=(ci == 0), stop=(ci == HC - 1))
                        nc.scalar.activation(
                            o_sb[:, co, :], ps, AF.Identity,
                            bias=b_sb[:, co:co + 1])
                v_sb = perb.tile([128, SB, H], BF16, tag="v")
                for sb_i in range(SB):
                    for half in range(2):
                        ps = psA.tile([128, S], F32, tag="ps", name="ps_v")
                        for ci in range(HC):
                            nc.tensor.matmul(
                                ps, hsT_sb[:, ci, sb_i * 128:(sb_i + 1) * 128],
                                wv_sb[:, ci, half * 512:(half + 1) * 512],
                                start=(ci == 0), stop=(ci == HC - 1))
                        nc.vector.tensor_add(
                            v_sb[:, sb_i, half * 512:(half + 1) * 512], ps,
                            bvb_sb[:, half * 512:(half + 1) * 512])

                # ---- attention ------------------------------------------------
                ctx_sb = perb.tile([128, SB, H], F32, tag="ctx")
                for h in range(NH):
                    hc, hp = h // 2, (h % 2) * 64
                    masked = work.tile([128, SB, S], F32, tag="masked", bufs=2,
                                       name="masked")
                    ssum = work.tile([128, SB], F32, tag="ssum", bufs=2, name="ssum")
                    ps_c = psC.tile([128, SB, DH], F32, tag="psc", name="ps_c")
                    for r in range(SB):
                        ps_s = psA.tile([128, S], F32, tag="ps", name="ps_s")
                        nc.tensor.matmul(
                            ps_s, qT_sb[hp:hp + 64, hc, r * 128:(r + 1) * 128],
                            kT_sb[hp:hp + 64, hc, :], start=True, stop=True)
                        nc.vector.tensor_add(masked[:, r, :], ps_s, m_sb[:, r, :])
                        p_sb = work.tile([128, S], BF16, tag="p", bufs=3, name="p_sb")
                        nc.scalar.activation(p_sb, masked[:, r, :], AF.Exp,
                                             accum_out=ssum[:, r:r + 1])
                        ps_t = psT.tile([128, S], BF16, tag="pst", name="ps_t")
                        for c in range(SB):
                            nc.tensor.transpose(
                                ps_t[:, c * 128:(c + 1) * 128],
                                p_sb[:, c * 128:(c + 1) * 128], ident_bf)
                        pgT = work.tile([128, SB, 128], BF16, tag="pgT", bufs=3,
                                        name="pgT")
                        nc.vector.tensor_mul(
                            pgT, ps_t.rearrange("p (c i) -> p c i", c=SB),
                            gpT_sb[:, :, r * 128:(r + 1) * 128])
                        for c in range(SB):
                            nc.tensor.matmul(
                                ps_c[:, r, :], pgT[:, c, :],
                                v_sb[:, c, h * DH:(h + 1) * DH],
                                start=(c == 0), stop=(c == SB - 1))
                    nc.sync.dma_start(
                        out=scores_d[b, h].rearrange("(r p) j -> p r j", p=128),
                        in_=masked)
                    rs = work.tile([128, SB], F32, tag="rs", bufs=2, name="rs")
                    nc.vector.reciprocal(rs, ssum)
                    for r in range(SB):
                        nc.scalar.mul(ctx_sb[:, r, h * DH:(h + 1) * DH],
                                      ps_c[:, r, :], rs[:, r:r + 1])
                nc.sync.dma_start(
                    out=ctx_d[b].rearrange("(r p) o -> p r o", p=128), in_=ctx_sb)
    nc.compile()
    return nc


_NC_CACHE = {}


def _get_nc():
    if "nc" not in _NC_CACHE:
        _NC_CACHE["nc"] = build_nc()
    return _NC_CACHE["nc"]


def prep_inputs(hidden_states, attention_mask, group_prob, Wq, bq, Wk, bk, Wv, bv):
    """Host-side layout prep + sharding. Returns list of 8 per-core input maps."""
    f = np.float32
    hs = np.asarray(hidden_states, dtype=f)
    hsT = np.ascontiguousarray(hs.transpose(0, 2, 1))
    wqT = np.ascontiguousarray(np.asarray(Wq, dtype=f).T / 8.0)
    wkT = np.ascontiguousarray(np.asarray(Wk, dtype=f).T)
    wvT = np.ascontiguousarray(np.asarray(Wv, dtype=f).T)
    bq8 = np.asarray(bq, dtype=f) / 8.0
    bk_ = np.asarray(bk, dtype=f)
    bv_ = np.asarray(bv, dtype=f)
    keep = (np.asarray(attention_mask)[:, 0] != 0) | np.eye(S, dtype=bool)
    madd = np.where(keep, 0, -np.inf).astype(ml_dtypes.bfloat16)
    gpT = np.ascontiguousarray(
        np.asarray(group_prob, dtype=f).transpose(0, 2, 1)).astype(ml_dtypes.bfloat16)
    in_maps = []
    for i in range(NCORES):
        sl = slice(i * NB, (i + 1) * NB)
        in_maps.append({
            "hsT": hsT[sl], "wqT": wqT, "wkT": wkT, "wvT": wvT,
            "bq": bq8, "bk": bk_, "bv": bv_,
            "madd": madd[sl], "gpT": gpT[sl],
        })
    return in_maps


def kernel(hidden_states, attention_mask, group_prob, Wq, bq, Wk, bk, Wv, bv):
    in_maps = prep_inputs(hidden_states, attention_mask, group_prob,
                          Wq, bq, Wk, bk, Wv, bv)
    nc = _get_nc()
    res = run_bass_kernel_spmd(nc, in_maps, core_ids=list(range(NCORES)))
    ctx = np.concatenate([res.results[i]["ctx"] for i in range(NCORES)], axis=0)
    scores = np.concatenate([res.results[i]["scores"] for i in range(NCORES)], axis=0)
    return ctx, scores
